# revision 22
# baseline (speedup 1.0000x reference)
"""BNN-MNIST forward pass as a hand-written Bass/Tile kernel, data-parallel
across 8 TRN2 NeuronCores (batch 1024 -> 128 images per core).

Numerical scheme (everything except conv1 is EXACT vs the fp32 reference):
  - conv1: weights are binarized (+-1, exact in bf16); x is split exactly into
    3 bf16 planes (x = hi + mid + lo). The 3x3 conv over 1 input channel is a
    single matmul with contraction = (parity 2, level 3, tap 9) = 54 rows and
    lhs free = (parity 2, out-channel 64) = 128 (block-diagonal weights), so
    two images are computed per streamed column. Products are exact; only the
    PE fp32 accumulation order differs from the CPU reference (ulp-level).
  - sign1 (bn+clip+binarize folded to sign(h + t1)): ACT Sign, output +-1 fp8.
  - maxpool1: DVE max on fp8 (+-1), after sign (sign commutes with max).
  - conv2: +-1 weights x +-1 activations in fp8, integer accumulation in PSUM
    (exact). Contraction = (parity 2, in-channel 64) block-diagonal, 9 tap
    matmuls accumulate in PSUM.
  - sign2: step(p - m2) per element on DVE (0/1 fp8, exact integer compare),
    maxpool2 on GpSimd (max of 0/1 == pooled step, monotone).
  - fc1: 0/1 activations vs +-1 weights in fp8, exact integer PSUM; the 0/1
    correction and bn3 threshold fold into thr3[o] = (m3' + K1[o])/2.
  - fc2: 0/1 activations, exact integer result J; host computes the exact
    affine fixup out = (2J - sum(wfc2b) + bfc2) * scale in fp32.
"""

import functools
import numpy as np
import ml_dtypes

import concourse.bass as bass
import concourse.tile as tile
from concourse import bacc, mybir
from concourse.bass_utils import run_bass_kernel_spmd

F32 = mybir.dt.float32
BF16 = mybir.dt.bfloat16
F8 = mybir.dt.float8e4

NP_BF16 = ml_dtypes.bfloat16
NP_F8 = ml_dtypes.float8_e4m3

EPS = 1e-5
N_CORES = 8
BPC = 128          # images per core
PAIRS = BPC // 2   # image pairs per core
TAPS = [(dy, dx) for dy in range(3) for dx in range(3)]


# ---------------------------------------------------------------------------
# Device kernel builder
# ---------------------------------------------------------------------------

def _build_nc():
    nc = bacc.Bacc("TRN2", target_bir_lowering=False, debug=False,
                   num_devices=N_CORES)

    xp = nc.declare_dram_parameter("xp", [3, 2, PAIRS, 900], BF16, isOutput=False)
    w1s = nc.declare_dram_parameter("w1s", [54, 128], BF16, isOutput=False)
    w2s = nc.declare_dram_parameter("w2s", [9, 128, 128], F8, isOutput=False)
    wfc1 = nc.declare_dram_parameter("wfc1r", [25, 128, 2048], F8, isOutput=False)
    wfc2 = nc.declare_dram_parameter("wfc2r", [16, 128, 10], BF16, isOutput=False)
    t1v = nc.declare_dram_parameter("t1v", [128, 1], F32, isOutput=False)
    m2v = nc.declare_dram_parameter("m2v", [128, 1], F32, isOutput=False)
    thr3 = nc.declare_dram_parameter("thr3", [128, 2048], F32, isOutput=False)
    outp = nc.declare_dram_parameter("out", [BPC, 10], F32, isOutput=True)

    a2t_d = nc.dram_tensor("a2t_d", [25, 128, 128], F8)
    xstage = nc.dram_tensor("xstage", [54, PAIRS, 840], BF16)

    with tile.TileContext(nc) as tc:
        _body(nc, tc, xp, w1s, w2s, wfc1, wfc2, t1v, m2v, thr3, outp,
              a2t_d, xstage)

    nc.compile()
    return nc


def _body(nc, tc, xp, w1s, w2s, wfc1, wfc2, t1v, m2v, thr3, outp, a2t_d,
          xstage):
    from contextlib import ExitStack
    with ExitStack() as ctx:
        consts = ctx.enter_context(tc.tile_pool(name="consts", bufs=1))
        xpool = ctx.enter_context(tc.tile_pool(name="xplanes", bufs=1))
        apool = ctx.enter_context(tc.tile_pool(name="acts", bufs=1))
        wpool = ctx.enter_context(tc.tile_pool(name="wfc1", bufs=1))
        s1pool = ctx.enter_context(tc.tile_pool(name="s1", bufs=3))
        vpool = ctx.enter_context(tc.tile_pool(name="vt", bufs=3))
        s2pool = ctx.enter_context(tc.tile_pool(name="s2", bufs=3))
        kpool = ctx.enter_context(tc.tile_pool(name="kxb", bufs=4))

        # ------------------------------------------------------------------
        # x-path DMAs first (conv1 is gated on these); bulk weights are
        # emitted later so the DMA queues serve the critical path first.
        #
        # x planes: partition p = g*27 + l*9 + tap; per-partition data is the
        # padded 30x30 image plane of one (level, parity), pre-shifted by the
        # tap offset. Two-hop load: DRAM->DRAM tap-shift staging (full DMA
        # bandwidth, no partition penalty), then partition-parallel SBUF
        # block loads.
        # ------------------------------------------------------------------
        w1t = consts.tile([54, 128], BF16)
        nc.sync.dma_start(w1t[:], w1s[:])
        t1t = consts.tile([128, 1], F32)
        nc.sync.dma_start(t1t[:], t1v[:])
        xpt = xpool.tile([54, PAIRS, 840], BF16)
        for g in range(2):
            for l in range(3):
                for ti, (dy, dx) in enumerate(TAPS):
                    p = g * 27 + l * 9 + ti
                    sh = dy * 30 + dx
                    nc.sync.dma_start(xstage[p, :, 0:838],
                                      xp[l, g, :, sh:sh + 838])
        for blk in range(16):
            s0 = 4 * blk
            nc.sync.dma_start(xpt[:, s0:s0 + 4, 0:838],
                              xstage[:, s0:s0 + 4, 0:838])

        w2t = consts.tile([128, 9, 128], F8)
        for t in range(9):
            nc.sync.dma_start(w2t[:, t, :], w2s[t])
        m2t = consts.tile([128, 1], F32)
        nc.sync.dma_start(m2t[:], m2v[:])

        a1pad = apool.tile([128, PAIRS, 256], F8)
        nc.gpsimd.memset(a1pad[:], 0)
        # s-major so pair is the contiguous axis (needed by the a2t_d DMA)
        a2 = apool.tile([128, 50, PAIRS], F8)
        nc.gpsimd.memset(a2[:], 0)

        # bulk fc weights: no deps, lower priority -> stream in during the
        # conv phases
        wfc1t = wpool.tile([128, 25, 2048], F8)
        for c in range(25):
            nc.sync.dma_start(wfc1t[:, c, :], wfc1[c])
        thr3t = consts.tile([128, 2048], F32)
        nc.sync.dma_start(thr3t[:], thr3[:])
        wfc2t = consts.tile([128, 16, 10], BF16)
        for ch in range(16):
            nc.sync.dma_start(wfc2t[:, ch, :], wfc2[ch])

        # ------------------------------------------------------------------
        # block 1: conv1 (bf16x3) -> sign -> maxpool
        # chunk = half image of one pair; group = 4 chunks = 2 pairs
        # ------------------------------------------------------------------
        with tc.tile_pool(name="cps1", bufs=2, space="PSUM") as cps1:
            for grp in range(32):
                ps = cps1.tile([128, 4, 512], F32)
                for k in range(4):
                    pr = grp * 2 + k // 2
                    h = k % 2
                    xv = xpt[:, pr, :].rearrange("p (y c) -> p y c", c=30)
                    nc.tensor.matmul(
                        ps[:, k, 0:392].rearrange("p (y x) -> p y x", x=28),
                        w1t[:],
                        xv[:, 14 * h:14 * h + 14, 0:28],
                        start=True, stop=True)
                # sign over the whole group in one ACT op: a1s = sign(h + t1)
                a1s = s1pool.tile([128, 4, 392], F8)
                nc.scalar.sign(a1s[:], ps[:, :, 0:392], bias=t1t[:])
                # maxpool on +-1 fp8: vertical then horizontal
                a1v = a1s[:].rearrange("p k (yo two x) -> p k yo two x",
                                       two=2, x=28)
                vt = vpool.tile([128, 4, 7, 28], F8)
                nc.vector.tensor_max(vt[:], a1v[:, :, :, 0, :],
                                     a1v[:, :, :, 1, :])
                vv = vt[:].rearrange(
                    "p (pr h) yo (xo two) -> p pr h yo xo two", h=2, two=2)
                av = a1pad[:, grp * 2:grp * 2 + 2, :].rearrange(
                    "p pr (r c) -> p pr r c", c=16)
                # pooled rows for half h land at a1pad rows 1+7h .. 8+7h
                dst = av[:, :, 1:15, 1:15].rearrange(
                    "p pr (h yo) xo -> p pr h yo xo", h=2)
                nc.vector.tensor_max(dst, vv[:, :, :, :, :, 0],
                                     vv[:, :, :, :, :, 1])

        # ------------------------------------------------------------------
        # block 2: conv2 (fp8, 9 taps accumulate) -> step -> maxpool
        # chunk = 2 pairs
        # ------------------------------------------------------------------
        with tc.tile_pool(name="cps2", bufs=2, space="PSUM") as cps2:
            for ch2 in range(32):
                ps = cps2.tile([128, 2, 14, 14], F32)
                base = a1pad[:, 2 * ch2:2 * ch2 + 2, :].rearrange(
                    "p pr (r c) -> p pr r c", c=16)
                for ti, (dy, dx) in enumerate(TAPS):
                    nc.tensor.matmul(ps[:], w2t[:, ti, :],
                                     base[:, :, dy:dy + 14, dx:dx + 14],
                                     start=(ti == 0), stop=(ti == 8))
                # elementwise step(p - m2) -> 0/1 fp8 (commutes with maxpool)
                a2s = s2pool.tile([128, 2, 14, 14], F8)
                nc.vector.tensor_scalar(a2s[:], ps[:], m2t[:], None,
                                        mybir.AluOpType.is_ge)
                # maxpool 0/1 on gpsimd
                a2v = a2s[:].rearrange("p pr (yo two) x -> p pr yo two x",
                                       two=2)
                vt2 = vpool.tile([128, 2, 7, 14], F8, tag="vt2")
                nc.vector.tensor_max(vt2[:], a2v[:, :, :, 0, :],
                                     a2v[:, :, :, 1, :])
                vv2 = vt2[:].rearrange("p pr yo (xo two) -> p pr yo xo two",
                                       two=2)
                dst2 = a2[:, 0:49, 2 * ch2:2 * ch2 + 2].rearrange(
                    "p (yo xo) pr -> p pr yo xo", xo=7)
                nc.vector.tensor_max(dst2, vv2[:, :, :, :, 0],
                                     vv2[:, :, :, :, 1])

        # ------------------------------------------------------------------
        # a2 [128=(g,ci), pair, 50] -> a2t_d[c][(2ci+s_sub), b=(2*pair+g)]
        # ------------------------------------------------------------------
        # column index of a2t_d is b' = g*64 + pair (host reorders rows of
        # the final output back to b = 2*pair + g)
        for c in range(25):
            for g in range(2):
                src = a2[64 * g:64 * g + 64, 2 * c:2 * c + 2, :]  # [64,2,64]
                dst = a2t_d[c].rearrange(
                    "(ci s) (g pair) -> g ci s pair", s=2, g=2)[g]
                nc.sync.dma_start(dst, src)

        # ------------------------------------------------------------------
        # fc1: 25 k-chunks accumulate into one 4-bank PSUM tile
        # ------------------------------------------------------------------
        with tc.tile_pool(name="fps", bufs=1, space="PSUM") as fps:
            psf = fps.tile([128, 2048], F32)
            for c in range(25):
                kt = kpool.tile([128, 128], F8)
                nc.sync.dma_start(kt[:], a2t_d[c])
                for oc in range(4):
                    nc.tensor.matmul(psf[:, 512 * oc:512 * oc + 512], kt[:],
                                     wfc1t[:, c, 512 * oc:512 * oc + 512],
                                     start=(c == 0), stop=(c == 24))
            a3 = apool.tile([128, 2048], BF16)
            nc.vector.tensor_tensor(a3[:], psf[:], thr3t[:],
                                    mybir.AluOpType.is_ge)

        # ------------------------------------------------------------------
        # fc2: 16 k-chunks accumulate; output raw integer J
        # a3 chunks transposed on the fly via the DMA XBAR (bf16)
        # ------------------------------------------------------------------
        with tc.tile_pool(name="ops", bufs=1, space="PSUM") as ops_:
            pso = ops_.tile([128, 10], F32)
            for ch in range(16):
                at = kpool.tile([128, 128], BF16, tag="a3t")
                nc.sync.dma_start(at[:], a3[:, 128 * ch:128 * ch + 128],
                                  transpose=True)
                nc.tensor.matmul(pso[:], at[:], wfc2t[:, ch, :],
                                 start=(ch == 0), stop=(ch == 15))
            outt = consts.tile([BPC, 10], F32, tag="outt")
            nc.scalar.copy(outt[:], pso[:])
            nc.sync.dma_start(outp[:], outt[:])


# ---------------------------------------------------------------------------
# Host-side prep
# ---------------------------------------------------------------------------

def _binarize(w):
    return np.where(np.asarray(w, np.float32) >= 0, 1.0, -1.0).astype(np.float32)


def _prep(x, w1, b1, g1, be1, m1, v1, w2, b2, g2, be2, m2, v2,
          wfc1, bfc1, g3, be3, m3, v3, wfc2, bfc2, scale):
    B = x.shape[0]
    w1b = _binarize(w1)          # [64,1,3,3]
    w2b = _binarize(w2)          # [64,64,3,3]
    wfc1b = _binarize(wfc1)      # [2048,3136]
    wfc2b = _binarize(wfc2)      # [10,2048]

    s1 = np.asarray(g1, np.float32) / np.sqrt(np.asarray(v1, np.float32) + EPS)
    s2 = np.asarray(g2, np.float32) / np.sqrt(np.asarray(v2, np.float32) + EPS)
    s3 = np.asarray(g3, np.float32) / np.sqrt(np.asarray(v3, np.float32) + EPS)
    # sign(h + t1) == sign(bn1(h)) since s1 > 0 (conv bias b1 folded in)
    t1 = (np.asarray(be1, np.float32) / s1 - np.asarray(m1, np.float32)
          + np.asarray(b1, np.float32)).astype(np.float32)
    # block2 threshold: p >= m2eff  (p = integer conv2 psum)
    m2eff = (np.asarray(m2, np.float32) - np.asarray(b2, np.float32)
             - np.asarray(be2, np.float32) / s2).astype(np.float32)
    # fc1 on 0/1 inputs: n_pm = 2*n01 - K1; condition n_pm >= m3eff
    m3eff = (np.asarray(m3, np.float32) - np.asarray(bfc1, np.float32)
             - np.asarray(be3, np.float32) / s3).astype(np.float32)
    K1 = wfc1b.sum(axis=1).astype(np.float32)          # [2048]
    thr3 = ((m3eff + K1) / 2.0).astype(np.float32)     # [2048]
    c2 = wfc2b.sum(axis=1).astype(np.float32)          # [10]

    # --- exact bf16x3 split of the padded input ---
    xs = np.asarray(x, np.float32).reshape(B, 28, 28)
    xpad = np.zeros((B, 30, 30), np.float32)
    xpad[:, 1:29, 1:29] = xs
    xh = xpad.astype(NP_BF16)
    r = xpad - xh.astype(np.float32)
    xm = r.astype(NP_BF16)
    xl = (r - xm.astype(np.float32)).astype(NP_BF16)
    planes = np.stack([xh, xm, xl])                    # [3, B, 30, 30] bf16
    planes = planes.reshape(3, B, 900)

    # conv1 stationary weights [54, 128] (block-diagonal over parity)
    w1sa = np.zeros((54, 128), np.float32)
    for g in range(2):
        for l in range(3):
            for ti, (dy, dx) in enumerate(TAPS):
                w1sa[g * 27 + l * 9 + ti, g * 64:g * 64 + 64] = w1b[:, 0, dy, dx]
    w1sa = w1sa.astype(NP_BF16)

    # conv2 weights [9, 128, 128] block-diagonal over parity
    w2sa = np.zeros((9, 128, 128), np.float32)
    for ti, (dy, dx) in enumerate(TAPS):
        blk = w2b[:, :, dy, dx].T                      # [ci, co]
        w2sa[ti, 0:64, 0:64] = blk
        w2sa[ti, 64:128, 64:128] = blk
    w2sa = w2sa.astype(NP_F8)

    # fc1 weights [25, 128, 2048]: row (2ci+s_sub) of chunk c = k (ci,2c+s_sub)
    Wp = np.zeros((2048, 64, 50), np.float32)
    Wp[:, :, :49] = wfc1b.reshape(2048, 64, 49)
    wfc1r = Wp.reshape(2048, 64, 25, 2).transpose(2, 1, 3, 0).reshape(
        25, 128, 2048).astype(NP_F8)
    wfc1r = np.ascontiguousarray(wfc1r)

    # fc2 weights [16, 128, 10]
    wfc2r = np.ascontiguousarray(
        wfc2b.T.reshape(16, 128, 10).astype(NP_BF16))

    t1v = np.concatenate([t1, t1]).reshape(128, 1).astype(np.float32)
    m2vv = np.concatenate([m2eff, m2eff]).reshape(128, 1).astype(np.float32)
    thr3bc = np.ascontiguousarray(
        np.broadcast_to(thr3[None, :], (128, 2048)).astype(np.float32))

    in_maps = []
    for c in range(N_CORES):
        pc = planes[:, c * BPC:(c + 1) * BPC]          # [3, 128, 900]
        xp_c = np.ascontiguousarray(
            np.stack([pc[:, 0::2], pc[:, 1::2]], axis=1))  # [3,2,64,900]
        in_maps.append({
            "xp": xp_c,
            "w1s": w1sa,
            "w2s": w2sa,
            "wfc1r": wfc1r,
            "wfc2r": wfc2r,
            "t1v": t1v,
            "m2v": m2vv,
            "thr3": thr3bc,
        })
    fixup = (c2, np.asarray(bfc2, np.float32), np.float32(np.asarray(scale)))
    return in_maps, fixup


# ---------------------------------------------------------------------------
# Cached compiled program + runner
# ---------------------------------------------------------------------------

_STATE = {}


def _get_runner():
    if "runner" in _STATE:
        return _STATE["runner"]
    nc = _build_nc()
    _STATE["nc"] = nc

    from concourse import bass2jax
    import jax

    bass2jax.install_neuronx_cc_hook()

    partition_name = (nc.partition_id_tensor.name
                      if nc.partition_id_tensor else None)
    in_names = []
    out_names = []
    out_avals = []
    zero_shapes = []
    for alloc in nc.m.functions[0].allocations:
        if not isinstance(alloc, mybir.MemoryLocationSet):
            continue
        name = alloc.memorylocations[0].name
        if alloc.kind == "ExternalInput":
            if name != partition_name:
                in_names.append(name)
        elif alloc.kind == "ExternalOutput":
            shape = tuple(alloc.tensor_shape)
            dtype = mybir.dt.np(alloc.dtype)
            out_names.append(name)
            out_avals.append(jax.core.ShapedArray(shape, dtype))
            zero_shapes.append((shape, dtype))
    n_params = len(in_names)
    n_outs = len(out_names)
    all_in_names = in_names + out_names
    if partition_name is not None:
        all_in_names = all_in_names + [partition_name]

    def _bodyfn(*args):
        operands = list(args)
        if partition_name is not None:
            operands.append(bass2jax.partition_id_tensor())
        outs = bass2jax._bass_exec_p.bind(
            *operands,
            out_avals=tuple(out_avals),
            in_names=tuple(all_in_names),
            out_names=tuple(out_names),
            lowering_input_output_aliases=(),
            sim_require_finite=True,
            sim_require_nnan=True,
            nc=nc,
        )
        return tuple(outs)

    from jax.sharding import Mesh, PartitionSpec
    from jax.experimental.shard_map import shard_map

    devices = jax.devices()[:N_CORES]
    mesh = Mesh(np.asarray(devices), ("core",))
    in_specs = (PartitionSpec("core"),) * (n_params + n_outs)
    out_specs = (PartitionSpec("core"),) * n_outs
    donate = tuple(range(n_params, n_params + n_outs))
    sharded = jax.jit(
        shard_map(_bodyfn, mesh=mesh, in_specs=in_specs,
                  out_specs=out_specs, check_rep=False),
        donate_argnums=donate, keep_unused=True)

    _STATE.update(dict(
        mesh=mesh, in_specs=in_specs, out_specs=out_specs,
        bodyfn=_bodyfn, in_names=in_names, zero_shapes=zero_shapes,
        n_params=n_params))

    def run(in_maps):
        per_core = [[np.asarray(m[nm]) for nm in in_names] for m in in_maps]
        concat_in = [
            np.concatenate([per_core[c][i] for c in range(N_CORES)], axis=0)
            for i in range(n_params)
        ]
        concat_zeros = [
            np.zeros((N_CORES * s[0], *s[1:]), d) for (s, d) in zero_shapes
        ]
        out_arrs = sharded(*concat_in, *concat_zeros)
        res = np.asarray(out_arrs[0]).reshape(N_CORES, BPC, 10)
        # device rows are ordered b' = g*64 + pair; restore b = 2*pair + g
        res = res.reshape(N_CORES, 2, PAIRS, 10).transpose(0, 2, 1, 3)
        return res.reshape(N_CORES, BPC, 10)

    _STATE["runner"] = run
    return run


def kernel(**inputs):
    in_maps, (c2, bfc2, scale) = _prep(**inputs)
    run = _get_runner()
    J = run(in_maps)                                   # [8, 128, 10] fp32
    J = J.reshape(N_CORES * BPC, 10)
    # exact integer fixup: h3@W = 2*J - c2 ; out = (I + bfc2) * scale in fp32
    I = (2.0 * J.astype(np.float64) - c2.astype(np.float64)).astype(np.float32)
    out = (I + bfc2[None, :]) * scale
    return out.astype(np.float32)


# expose in_maps/nc for the test harness (profiling path)
def _debug_handles(inputs):
    in_maps, fixup = _prep(**inputs)
    nc = _STATE.get("nc")
    if nc is None:
        _get_runner()
        nc = _STATE["nc"]
    return nc, in_maps, fixup


def _timed_exec(in_maps, iters=32):
    """Measure per-execution device time by queueing `iters` async
    executions of the NEFF with device-resident inputs (non-donating jit,
    so all buffers stay put) and timing tail-to-tail."""
    import time
    import jax
    from jax.experimental.shard_map import shard_map

    _get_runner()
    mesh = _STATE["mesh"]
    in_names = _STATE["in_names"]
    zero_shapes = _STATE["zero_shapes"]
    n_params = _STATE["n_params"]
    from jax.sharding import NamedSharding, PartitionSpec

    fn = jax.jit(
        shard_map(_STATE["bodyfn"], mesh=mesh, in_specs=_STATE["in_specs"],
                  out_specs=_STATE["out_specs"], check_rep=False),
        keep_unused=True)

    per_core = [[np.asarray(m[nm]) for nm in in_names] for m in in_maps]
    concat_in = [
        np.concatenate([per_core[c][i] for c in range(N_CORES)], axis=0)
        for i in range(n_params)
    ]
    concat_zeros = [
        np.zeros((N_CORES * s[0], *s[1:]), d) for (s, d) in zero_shapes
    ]
    sh = NamedSharding(mesh, PartitionSpec("core"))
    dev_in = [jax.device_put(a, sh) for a in concat_in]
    dev_zero = [jax.device_put(a, sh) for a in concat_zeros]

    out = fn(*dev_in, *dev_zero)
    jax.block_until_ready(out)
    # warm pass then timed async batches
    best = float("inf")
    for _ in range(3):
        t0 = time.perf_counter()
        outs = [fn(*dev_in, *dev_zero) for _ in range(iters)]
        jax.block_until_ready(outs)
        t1 = time.perf_counter()
        best = min(best, (t1 - t0) / iters)
    return best
